# revision 17
# baseline (speedup 1.0000x reference)
"""Trainium2 Bass kernel for nn_CrinaSynapse (sparse_attention).

Math notes (exact, not approximations):
  - The reference's attention softmax is over a size-1 axis, so the
    attention weights are exactly 1.0 and attn_out[b,h,s,:] == tree_v[b,h,:]
    for every s; the q.k scores never influence the output.
  - Therefore: out = LayerNorm(query + broadcast(tree_v)) * gamma + beta,
    where tree_v is the 15-node binary-tree reduction over the first 8
    modalities of key_value.

Sharding: data-parallel over batch — batch 8 == 8 NeuronCores, one batch
element per core. The tiny per-node 64x64 weights / LIF params are
replicated (host-transposed into kernel layout).

Device layout:
  - Tree phase in "transposed" layout: head_dim d on partitions (64),
    node*head on the free axis. Each node matmul is then
    out[e, h] = sum_d W[n,e,d] * x[d, h] with lhsT = W[n].T (host-prepped)
    and rhs the running activation — no per-level transposes needed.
  - tree_v root [64e, 16h] is PE-transposed to [16h, 64e] and broadcast to
    a [128, 1024] SBUF tile via 16 K=1 matmuls against a ones vector.
  - Main loop streams query in [128, 1024] tiles: vector add + bn_stats/
    bn_aggr layernorm stats, scalar-engine (x*rstd - mu*rstd) apply.
"""

import os
import sys

for _p in ("/opt/trn_rl_repo", "/root/.axon_site/_ro/trn_rl_repo"):
    if os.path.isdir(_p) and _p not in sys.path:
        sys.path.append(_p)

import numpy as np

import concourse.bass as bass
import concourse.tile as tile
from concourse import bacc, mybir
from concourse.bass_utils import run_bass_kernel_spmd
from concourse.tile_rust import add_dep_helper

F32 = mybir.dt.float32
AF = mybir.ActivationFunctionType
OP = mybir.AluOpType

BATCH = 8
SEQ = 4096
DM = 1024
NH = 16
HD = 64
P = 128
NT = SEQ // P  # 32 row-tiles per core
EPS = 1e-5
N_CORES = 8

# (psum-bank-safe) free-axis offsets of each tree level inside the packed
# [64, 240] parameter tensors: leaves(8 nodes)=0, lvl2(4)=128, lvl1(2)=192,
# root(1)=224. Node n occupies 16 columns per head.
LEVEL_OFF = {3: 0, 2: 128, 1: 192, 0: 224}


def _build_program(apply_affine: bool):
    nc = bacc.Bacc("TRN2", target_bir_lowering=False, debug=False,
                   num_devices=N_CORES)

    q = nc.dram_tensor("q", [SEQ, DM], F32, kind="ExternalInput")
    xT = nc.dram_tensor("xT", [HD, 128], F32, kind="ExternalInput")
    wT = nc.dram_tensor("wT", [HD, 15 * HD], F32, kind="ExternalInput")
    thT = nc.dram_tensor("thT", [HD, 240], F32, kind="ExternalInput")
    tauT = nc.dram_tensor("tauT", [HD, 240], F32, kind="ExternalInput")
    vrT = nc.dram_tensor("vrT", [HD, 240], F32, kind="ExternalInput")
    bT = nc.dram_tensor("bT", [HD, 240], F32, kind="ExternalInput")
    id64 = nc.dram_tensor("id64", [HD, HD], F32, kind="ExternalInput")
    if apply_affine:
        gam = nc.dram_tensor("gam", [DM], F32, kind="ExternalInput")
        bet = nc.dram_tensor("bet", [DM], F32, kind="ExternalInput")
    out = nc.dram_tensor("out", [SEQ, DM], F32, kind="ExternalOutput")
    tv_hbm = nc.dram_tensor("tv_scratch", [DM], F32, kind="Internal")

    with tile.TileContext(nc) as tc:
        with (
            tc.tile_pool(name="const", bufs=1) as const,
            tc.tile_pool(name="tree", bufs=2) as tree,
            tc.tile_pool(name="tpsum", bufs=2, space="PSUM") as tpsum,
            tc.tile_pool(name="qin", bufs=8) as qin,
            tc.tile_pool(name="outp", bufs=6) as outp,
            tc.tile_pool(name="stats", bufs=16) as stats,
        ):
            # ---- constants (HWDGE; issued before the q stream in priority)
            xT_s = const.tile([HD, 128], F32)
            nc.sync.dma_start(out=xT_s[:, :], in_=xT[:, :])
            ws = const.tile([HD, 15 * HD], F32)
            nc.sync.dma_start(out=ws[:, :], in_=wT[:, :])
            th_s = const.tile([HD, 240], F32)
            nc.sync.dma_start(out=th_s[:, :], in_=thT[:, :])
            tau_s = const.tile([HD, 240], F32)
            nc.sync.dma_start(out=tau_s[:, :], in_=tauT[:, :])
            vr_s = const.tile([HD, 240], F32)
            nc.sync.dma_start(out=vr_s[:, :], in_=vrT[:, :])
            b_s = const.tile([HD, 240], F32)
            nc.sync.dma_start(out=b_s[:, :], in_=bT[:, :])
            id_s = const.tile([HD, HD], F32)
            nc.sync.dma_start(out=id_s[:, :], in_=id64[:, :])
            eps_s = const.tile([P, 1], F32)
            nc.vector.memset(eps_s[:, :], EPS)

            # ---- leaf level (nodes 7..14): x feeds both LIF calls
            proj_p = tpsum.tile([HD, 128], F32)
            for n in range(8):
                nc.tensor.matmul(
                    proj_p[:, n * 16:(n + 1) * 16],
                    lhsT=ws[:, (7 + n) * 64:(8 + n) * 64],
                    rhs=xT_s[:, n * 16:(n + 1) * 16],
                    start=True, stop=True,
                )
            proj = tree.tile([HD, 128], F32)
            nc.vector.tensor_add(proj[:, :], proj_p[:, :], b_s[:, 0:128])
            sk = tree.tile([HD, 128], F32)
            nc.vector.tensor_tensor(sk[:, :], xT_s[:, :], th_s[:, 0:128], OP.is_ge)
            tmp = tree.tile([HD, 128], F32)
            nc.vector.tensor_sub(tmp[:, :], vr_s[:, 0:128], xT_s[:, :])
            nc.vector.tensor_mul(tmp[:, :], tmp[:, :], sk[:, :])
            nc.vector.tensor_add(tmp[:, :], xT_s[:, :], tmp[:, :])  # s1
            v2 = tree.tile([HD, 128], F32)
            nc.vector.tensor_mul(v2[:, :], tau_s[:, 0:128], tmp[:, :])
            nc.vector.tensor_add(v2[:, :], v2[:, :], xT_s[:, :])
            sv = tree.tile([HD, 128], F32)
            nc.vector.tensor_tensor(sv[:, :], v2[:, :], th_s[:, 0:128], OP.is_ge)
            ok = tree.tile([HD, 128], F32)
            nc.vector.tensor_mul(ok[:, :], proj[:, :], sk[:, :])
            ov = tree.tile([HD, 128], F32)
            nc.vector.tensor_mul(ov[:, :], proj[:, :], sv[:, :])

            # ---- inner levels. fk/fv here are the raw pair SUMS: the 0.5 of
            # the reference's pair-mean is folded into host-prescaled weights
            # (W_inner*0.5) and LIF params (th_inner*2, vr_inner*2) — exact
            # power-of-two rescalings, so spikes/projections are bit-identical.
            for level, size in [(2, 4), (1, 2), (0, 1)]:
                w = size * 16
                o = LEVEL_OFF[level]
                start = (1 << level) - 1
                pk = ok[:, :].rearrange("d (s r) -> d s r", r=32)
                pv = ov[:, :].rearrange("d (s r) -> d s r", r=32)
                fk = tree.tile([HD, w], F32)
                fv = tree.tile([HD, w], F32)
                fk3 = fk[:, :].rearrange("d (s h) -> d s h", h=16)
                fv3 = fv[:, :].rearrange("d (s h) -> d s h", h=16)
                nc.vector.tensor_add(fk3, pk[:, :, 0:16], pk[:, :, 16:32])
                nc.vector.tensor_add(fv3, pv[:, :, 0:16], pv[:, :, 16:32])

                nkp = tpsum.tile([HD, w], F32)
                nvp = tpsum.tile([HD, w], F32)
                for i in range(size):
                    m = start + i
                    nc.tensor.matmul(
                        nkp[:, i * 16:(i + 1) * 16],
                        lhsT=ws[:, m * 64:(m + 1) * 64],
                        rhs=fk[:, i * 16:(i + 1) * 16],
                        start=True, stop=True,
                    )
                    nc.tensor.matmul(
                        nvp[:, i * 16:(i + 1) * 16],
                        lhsT=ws[:, m * 64:(m + 1) * 64],
                        rhs=fv[:, i * 16:(i + 1) * 16],
                        start=True, stop=True,
                    )
                nk = tree.tile([HD, w], F32)
                nc.vector.tensor_add(nk[:, :], nkp[:, :], b_s[:, o:o + w])
                nv = tree.tile([HD, w], F32)
                nc.vector.tensor_add(nv[:, :], nvp[:, :], b_s[:, o:o + w])
                # LIF pair: xk=fk, xv=fv, shared membrane state
                sk = tree.tile([HD, w], F32)
                nc.vector.tensor_tensor(sk[:, :], fk[:, :], th_s[:, o:o + w], OP.is_ge)
                tmp = tree.tile([HD, w], F32)
                nc.vector.tensor_sub(tmp[:, :], vr_s[:, o:o + w], fk[:, :])
                nc.vector.tensor_mul(tmp[:, :], tmp[:, :], sk[:, :])
                nc.vector.tensor_add(tmp[:, :], fk[:, :], tmp[:, :])  # s1
                v2 = tree.tile([HD, w], F32)
                nc.vector.tensor_mul(v2[:, :], tau_s[:, o:o + w], tmp[:, :])
                nc.vector.tensor_add(v2[:, :], v2[:, :], fv[:, :])
                sv = tree.tile([HD, w], F32)
                nc.vector.tensor_tensor(sv[:, :], v2[:, :], th_s[:, o:o + w], OP.is_ge)
                ok = tree.tile([HD, w], F32)
                nc.vector.tensor_mul(ok[:, :], nk[:, :], sk[:, :])
                ov = tree.tile([HD, w], F32)
                nc.vector.tensor_mul(ov[:, :], nv[:, :], sv[:, :])

            # ---- root ov [64e, 16h] -> natural [16h, 64e] -> tvb [128, 1024]
            ovn_p = tpsum.tile([NH, HD], F32)
            nc.tensor.transpose(ovn_p[:, :], ov[:, :], id_s[:, :])
            ovn = tree.tile([NH, HD], F32)
            nc.scalar.copy(ovn[:, :], ovn_p[:, :])

            # round-trip tree_v through DRAM so a stride-0-partition DMA can
            # replicate the [1024] vector onto all 128 partitions
            # scalar-engine HWDGE: fast trigger/completion, and the only things
            # behind it in the scalar stream (first SQRT) wait on this anyway
            tvb = const.tile([P, DM], F32)
            wdma = nc.scalar.dma_start(
                out=tv_hbm[:].rearrange("(h e) -> h e", e=HD), in_=ovn[:, :])
            tvf = tv_hbm[:]
            rdma = nc.scalar.dma_start(
                out=tvb[:, :],
                in_=bass.AP(tensor=tvf.tensor, offset=tvf.offset,
                            ap=[[0, P], *tvf.ap]),
            )
            add_dep_helper(rdma.ins, wdma.ins,
                           reason="tvb broadcast reads tv_scratch after write")

            if apply_affine:
                gb = const.tile([P, DM], F32)
                bb = const.tile([P, DM], F32)
                g_ap = gam[:]
                nc.gpsimd.dma_start(
                    out=gb[:, :],
                    in_=bass.AP(tensor=g_ap.tensor, offset=g_ap.offset,
                                ap=[[0, P], *g_ap.ap]),
                )
                b_ap = bet[:]
                nc.gpsimd.dma_start(
                    out=bb[:, :],
                    in_=bass.AP(tensor=b_ap.tensor, offset=b_ap.offset,
                                ap=[[0, P], *b_ap.ap]),
                )

            # ---- main loop: res = q + tvb; layernorm over the 1024 free dim.
            # 2 row-tiles per DMA block to halve trigger/semaphore pressure.
            TPB = 2  # tiles per block
            qv = q[:, :].rearrange("(n p) d -> p n d", p=P)    # [128, 32, 1024]
            ovw = out[:, :].rearrange("(n p) d -> p n d", p=P)
            for i in range(NT // TPB):
                qt = qin.tile([P, TPB, DM], F32)
                nc.sync.dma_start(out=qt[:, :, :],
                                  in_=qv[:, i * TPB:(i + 1) * TPB, :])
                ot = outp.tile([P, TPB, DM], F32)
                for j in range(TPB):
                    sub = qt[:, j, :]
                    nc.vector.tensor_add(sub, sub, tvb[:, :])
                    st = stats.tile([P, 2, 6], F32)
                    nc.vector.bn_stats(st[:, 0, :], qt[:, j, 0:512])
                    nc.vector.bn_stats(st[:, 1, :], qt[:, j, 512:1024])
                    mv = stats.tile([P, 2], F32)
                    nc.vector.bn_aggr(mv[:, :], st[:, :, :])
                    istd = stats.tile([P, 1], F32)
                    nc.scalar.activation(istd[:, :], mv[:, 1:2], AF.Sqrt,
                                         bias=eps_s[:, :], scale=1.0)
                    nc.vector.reciprocal(istd[:, :], istd[:, :])
                    nmu = stats.tile([P, 1], F32)
                    nc.vector.tensor_scalar(nmu[:, :], mv[:, 0:1], istd[:, :],
                                            -1.0, OP.mult, OP.mult)
                    nc.scalar.activation(ot[:, j, :], sub, AF.Identity,
                                         bias=nmu[:, :], scale=istd[:, :])
                    if apply_affine:
                        nc.vector.tensor_mul(ot[:, j, :], ot[:, j, :], gb[:, :])
                        nc.vector.tensor_add(ot[:, j, :], ot[:, j, :], bb[:, :])
                # out-DMA triggered from the scalar sequencer so the in/out
                # trigger streams don't serialize on one sequencer
                nc.scalar.dma_start(out=ovw[:, i * TPB:(i + 1) * TPB, :],
                                    in_=ot[:, :, :])

    nc.compile()
    return nc


_PROGRAM_CACHE = {}


def _get_program(apply_affine: bool):
    if apply_affine not in _PROGRAM_CACHE:
        _PROGRAM_CACHE[apply_affine] = _build_program(apply_affine)
    return _PROGRAM_CACHE[apply_affine]


def _pack_params(p):
    """[15, 64] per-node params -> [64, 240] kernel layout (levels packed,
    replicated across the 16 heads)."""
    parts = []
    for start, size in [(7, 8), (3, 4), (1, 2), (0, 1)]:
        parts.append(np.repeat(p[start:start + size].T, NH, axis=1))
    return np.ascontiguousarray(np.concatenate(parts, axis=1), dtype=np.float32)


def make_in_maps(query, key_value, W, bias, lif_th, lif_tau, lif_vreset,
                 ln_gamma, ln_beta, apply_affine):
    # fold the tree's pair-mean 0.5 into the inner-node (0..6) weights and
    # LIF thresholds/resets — exact power-of-two rescalings (see kernel)
    W2 = W.astype(np.float32).copy()
    W2[0:7] *= 0.5
    th2 = lif_th.astype(np.float32).copy()
    th2[0:7] *= 2.0
    vr2 = lif_vreset.astype(np.float32).copy()
    vr2[0:7] *= 2.0
    wT = np.ascontiguousarray(W2.transpose(2, 0, 1).reshape(HD, 15 * HD))
    thT = _pack_params(th2)
    tauT = _pack_params(lif_tau.astype(np.float32))
    vrT = _pack_params(vr2)
    bT = _pack_params(bias.astype(np.float32))
    id64 = np.eye(HD, dtype=np.float32)
    in_maps = []
    for b in range(BATCH):
        xT = np.ascontiguousarray(
            key_value[b].astype(np.float32).reshape(16, NH, HD)[:8]
            .transpose(2, 0, 1).reshape(HD, 128))
        m = dict(q=np.ascontiguousarray(query[b], dtype=np.float32),
                 xT=xT, wT=wT, thT=thT, tauT=tauT, vrT=vrT, bT=bT, id64=id64)
        if apply_affine:
            m["gam"] = np.ascontiguousarray(ln_gamma, dtype=np.float32)
            m["bet"] = np.ascontiguousarray(ln_beta, dtype=np.float32)
        in_maps.append(m)
    return in_maps


def kernel(query, key_value, W, bias, lif_th, lif_tau, lif_vreset,
           ln_gamma, ln_beta, **run_kwargs):
    query = np.asarray(query, dtype=np.float32)
    key_value = np.asarray(key_value, dtype=np.float32)
    W = np.asarray(W, dtype=np.float32)
    bias = np.asarray(bias, dtype=np.float32)
    lif_th = np.asarray(lif_th, dtype=np.float32)
    lif_tau = np.asarray(lif_tau, dtype=np.float32)
    lif_vreset = np.asarray(lif_vreset, dtype=np.float32)
    ln_gamma = np.asarray(ln_gamma, dtype=np.float32)
    ln_beta = np.asarray(ln_beta, dtype=np.float32)

    apply_affine = not (np.all(ln_gamma == 1.0) and np.all(ln_beta == 0.0))
    nc = _get_program(apply_affine)
    in_maps = make_in_maps(query, key_value, W, bias, lif_th, lif_tau,
                           lif_vreset, ln_gamma, ln_beta, apply_affine)
    res = run_bass_kernel_spmd(nc, in_maps, core_ids=list(range(N_CORES)),
                               **run_kwargs)
    global _LAST_RESULTS
    _LAST_RESULTS = res
    out = np.stack([res.results[b]["out"] for b in range(BATCH)], axis=0)
    return out


_LAST_RESULTS = None


# revision 23
# speedup vs baseline: 1.0595x; 1.0595x over previous
"""Trainium2 Bass kernel for nn_CrinaSynapse (sparse_attention).

Math notes (exact, not approximations):
  - The reference's attention softmax is over a size-1 axis, so the
    attention weights are exactly 1.0 and attn_out[b,h,s,:] == tree_v[b,h,:]
    for every s; the q.k scores never influence the output.
  - Therefore: out = LayerNorm(query + broadcast(tree_v)) * gamma + beta,
    where tree_v is the 15-node binary-tree reduction over the first 8
    modalities of key_value.

Sharding: data-parallel over batch — batch 8 == 8 NeuronCores, one batch
element per core. The tiny per-node 64x64 weights / LIF params are
replicated (host-transposed into kernel layout).

Device layout:
  - Tree phase in "transposed" layout: head_dim d on partitions (64),
    node*head on the free axis. Each node matmul is then
    out[e, h] = sum_d W[n,e,d] * x[d, h] with lhsT = W[n].T (host-prepped)
    and rhs the running activation — no per-level transposes needed.
  - tree_v root [64e, 16h] is PE-transposed to [16h, 64e] and broadcast to
    a [128, 1024] SBUF tile via 16 K=1 matmuls against a ones vector.
  - Main loop streams query in [128, 1024] tiles: vector add + bn_stats/
    bn_aggr layernorm stats, scalar-engine (x*rstd - mu*rstd) apply.
"""

import os
import sys

for _p in ("/opt/trn_rl_repo", "/root/.axon_site/_ro/trn_rl_repo"):
    if os.path.isdir(_p) and _p not in sys.path:
        sys.path.append(_p)

import numpy as np

import concourse.bass as bass
import concourse.tile as tile
from concourse import bacc, mybir
from concourse.bass_utils import run_bass_kernel_spmd
from concourse.tile_rust import add_dep_helper

F32 = mybir.dt.float32
AF = mybir.ActivationFunctionType
OP = mybir.AluOpType

BATCH = 8
SEQ = 4096
DM = 1024
NH = 16
HD = 64
P = 128
NT = SEQ // P  # 32 row-tiles per core
EPS = 1e-5
N_CORES = 8

# (psum-bank-safe) free-axis offsets of each tree level inside the packed
# [64, 240] parameter tensors: leaves(8 nodes)=0, lvl2(4)=128, lvl1(2)=192,
# root(1)=224. Node n occupies 16 columns per head.
LEVEL_OFF = {3: 0, 2: 128, 1: 192, 0: 224}


def _build_program(apply_affine: bool):
    nc = bacc.Bacc("TRN2", target_bir_lowering=False, debug=False,
                   num_devices=N_CORES)

    q = nc.dram_tensor("q", [SEQ, DM], F32, kind="ExternalInput")
    xT = nc.dram_tensor("xT", [HD, 128], F32, kind="ExternalInput")
    wT = nc.dram_tensor("wT", [HD, 15 * HD], F32, kind="ExternalInput")
    thT = nc.dram_tensor("thT", [HD, 240], F32, kind="ExternalInput")
    tauT = nc.dram_tensor("tauT", [HD, 240], F32, kind="ExternalInput")
    vrT = nc.dram_tensor("vrT", [HD, 240], F32, kind="ExternalInput")
    bT = nc.dram_tensor("bT", [HD, 240], F32, kind="ExternalInput")
    id64 = nc.dram_tensor("id64", [HD, HD], F32, kind="ExternalInput")
    sel = nc.dram_tensor("sel", [NH, NH * P], F32, kind="ExternalInput")
    if apply_affine:
        gam = nc.dram_tensor("gam", [DM], F32, kind="ExternalInput")
        bet = nc.dram_tensor("bet", [DM], F32, kind="ExternalInput")
    out = nc.dram_tensor("out", [SEQ, DM], F32, kind="ExternalOutput")

    with tile.TileContext(nc) as tc:
        with (
            tc.tile_pool(name="const", bufs=1) as const,
            tc.tile_pool(name="tree", bufs=2) as tree,
            tc.tile_pool(name="tpsum", bufs=1, space="PSUM") as tpsum,
            tc.tile_pool(name="qin", bufs=8) as qin,
            tc.tile_pool(name="outp", bufs=6) as outp,
            tc.tile_pool(name="stats", bufs=16) as stats,
        ):
            # ---- constants (HWDGE; issued before the q stream in priority)
            xT_s = const.tile([HD, 128], F32)
            nc.sync.dma_start(out=xT_s[:, :], in_=xT[:, :])
            ws = const.tile([HD, 15 * HD], F32)
            nc.sync.dma_start(out=ws[:, :], in_=wT[:, :])
            th_s = const.tile([HD, 240], F32)
            nc.sync.dma_start(out=th_s[:, :], in_=thT[:, :])
            tau_s = const.tile([HD, 240], F32)
            nc.sync.dma_start(out=tau_s[:, :], in_=tauT[:, :])
            vr_s = const.tile([HD, 240], F32)
            nc.sync.dma_start(out=vr_s[:, :], in_=vrT[:, :])
            b_s = const.tile([HD, 240], F32)
            nc.sync.dma_start(out=b_s[:, :], in_=bT[:, :])
            id_s = const.tile([HD, HD], F32)
            nc.sync.dma_start(out=id_s[:, :], in_=id64[:, :])
            sel_s = const.tile([NH, NH * P], F32)
            nc.sync.dma_start(out=sel_s[:, :], in_=sel[:, :])
            eps_s = const.tile([P, 1], F32)
            nc.vector.memset(eps_s[:, :], EPS)

            # ---- leaf level (nodes 7..14): x feeds both LIF calls
            proj_p = tpsum.tile([HD, 128], F32)
            for n in range(8):
                nc.tensor.matmul(
                    proj_p[:, n * 16:(n + 1) * 16],
                    lhsT=ws[:, (7 + n) * 64:(8 + n) * 64],
                    rhs=xT_s[:, n * 16:(n + 1) * 16],
                    start=True, stop=True,
                )
            proj = tree.tile([HD, 128], F32)
            nc.vector.tensor_add(proj[:, :], proj_p[:, :], b_s[:, 0:128])
            sk = tree.tile([HD, 128], F32)
            nc.vector.tensor_tensor(sk[:, :], xT_s[:, :], th_s[:, 0:128], OP.is_ge)
            tmp = tree.tile([HD, 128], F32)
            nc.vector.tensor_sub(tmp[:, :], vr_s[:, 0:128], xT_s[:, :])
            nc.vector.tensor_mul(tmp[:, :], tmp[:, :], sk[:, :])
            nc.vector.tensor_add(tmp[:, :], xT_s[:, :], tmp[:, :])  # s1
            v2 = tree.tile([HD, 128], F32)
            nc.vector.tensor_mul(v2[:, :], tau_s[:, 0:128], tmp[:, :])
            nc.vector.tensor_add(v2[:, :], v2[:, :], xT_s[:, :])
            sv = tree.tile([HD, 128], F32)
            nc.vector.tensor_tensor(sv[:, :], v2[:, :], th_s[:, 0:128], OP.is_ge)
            ok = tree.tile([HD, 128], F32)
            nc.vector.tensor_mul(ok[:, :], proj[:, :], sk[:, :])
            ov = tree.tile([HD, 128], F32)
            nc.vector.tensor_mul(ov[:, :], proj[:, :], sv[:, :])

            # ---- inner levels. fk/fv here are the raw pair SUMS: the 0.5 of
            # the reference's pair-mean is folded into host-prescaled weights
            # (W_inner*0.5) and LIF params (th_inner*2, vr_inner*2) — exact
            # power-of-two rescalings, so spikes/projections are bit-identical.
            for level, size in [(2, 4), (1, 2), (0, 1)]:
                w = size * 16
                o = LEVEL_OFF[level]
                start = (1 << level) - 1
                pk = ok[:, :].rearrange("d (s r) -> d s r", r=32)
                pv = ov[:, :].rearrange("d (s r) -> d s r", r=32)
                fk = tree.tile([HD, w], F32)
                fv = tree.tile([HD, w], F32)
                fk3 = fk[:, :].rearrange("d (s h) -> d s h", h=16)
                fv3 = fv[:, :].rearrange("d (s h) -> d s h", h=16)
                nc.vector.tensor_add(fk3, pk[:, :, 0:16], pk[:, :, 16:32])
                nc.vector.tensor_add(fv3, pv[:, :, 0:16], pv[:, :, 16:32])

                nkp = tpsum.tile([HD, w], F32)
                nvp = tpsum.tile([HD, w], F32)
                for i in range(size):
                    m = start + i
                    nc.tensor.matmul(
                        nkp[:, i * 16:(i + 1) * 16],
                        lhsT=ws[:, m * 64:(m + 1) * 64],
                        rhs=fk[:, i * 16:(i + 1) * 16],
                        start=True, stop=True,
                    )
                    nc.tensor.matmul(
                        nvp[:, i * 16:(i + 1) * 16],
                        lhsT=ws[:, m * 64:(m + 1) * 64],
                        rhs=fv[:, i * 16:(i + 1) * 16],
                        start=True, stop=True,
                    )
                nk = tree.tile([HD, w], F32)
                nc.vector.tensor_add(nk[:, :], nkp[:, :], b_s[:, o:o + w])
                nv = tree.tile([HD, w], F32)
                nc.vector.tensor_add(nv[:, :], nvp[:, :], b_s[:, o:o + w])
                # LIF pair: xk=fk, xv=fv, shared membrane state
                sk = tree.tile([HD, w], F32)
                nc.vector.tensor_tensor(sk[:, :], fk[:, :], th_s[:, o:o + w], OP.is_ge)
                tmp = tree.tile([HD, w], F32)
                nc.vector.tensor_sub(tmp[:, :], vr_s[:, o:o + w], fk[:, :])
                nc.vector.tensor_mul(tmp[:, :], tmp[:, :], sk[:, :])
                nc.vector.tensor_add(tmp[:, :], fk[:, :], tmp[:, :])  # s1
                v2 = tree.tile([HD, w], F32)
                nc.vector.tensor_mul(v2[:, :], tau_s[:, o:o + w], tmp[:, :])
                nc.vector.tensor_add(v2[:, :], v2[:, :], fv[:, :])
                sv = tree.tile([HD, w], F32)
                nc.vector.tensor_tensor(sv[:, :], v2[:, :], th_s[:, o:o + w], OP.is_ge)
                ok = tree.tile([HD, w], F32)
                nc.vector.tensor_mul(ok[:, :], nk[:, :], sk[:, :])
                ov = tree.tile([HD, w], F32)
                nc.vector.tensor_mul(ov[:, :], nv[:, :], sv[:, :])

            # ---- root ov [64e, 16h] -> natural [16h, 64e] -> tvb [128, 1024]
            ovn_p = tpsum.tile([NH, HD], F32)
            nc.tensor.transpose(ovn_p[:, :], ov[:, :], id_s[:, :])
            ovn = tree.tile([NH, HD], F32)
            nc.scalar.copy(ovn[:, :], ovn_p[:, :])

            # round-trip tree_v through DRAM so a stride-0-partition DMA can
            # replicate the [1024] vector onto all 128 partitions
            # broadcast tree_v across all 128 partitions on the (idle) PE:
            # tvb[p, h*64+e] = sum_k sel_h[k, p] * ovn[k, e] with sel_h the
            # one-hot-row selector — no DRAM round-trip, no DMA latency
            tvb = const.tile([P, DM], F32)
            tvb_p0 = tpsum.tile([P, 512], F32)
            tvb_p1 = tpsum.tile([P, 512], F32)
            for h in range(NH):
                dst = tvb_p0 if h < 8 else tvb_p1
                c = (h % 8) * 64
                nc.tensor.matmul(
                    dst[:, c:c + 64],
                    lhsT=sel_s[:, h * P:(h + 1) * P],
                    rhs=ovn[:, :],
                    start=True, stop=True,
                )
            nc.vector.tensor_copy(tvb[:, 0:512], tvb_p0[:, :])
            nc.vector.tensor_copy(tvb[:, 512:1024], tvb_p1[:, :])

            if apply_affine:
                gb = const.tile([P, DM], F32)
                bb = const.tile([P, DM], F32)
                g_ap = gam[:]
                nc.gpsimd.dma_start(
                    out=gb[:, :],
                    in_=bass.AP(tensor=g_ap.tensor, offset=g_ap.offset,
                                ap=[[0, P], *g_ap.ap]),
                )
                b_ap = bet[:]
                nc.gpsimd.dma_start(
                    out=bb[:, :],
                    in_=bass.AP(tensor=b_ap.tensor, offset=b_ap.offset,
                                ap=[[0, P], *b_ap.ap]),
                )

            # ---- main loop: res = q + tvb; layernorm over the 1024 free dim.
            # 2 row-tiles per DMA block to halve trigger/semaphore pressure.
            TPB = 2  # tiles per block
            qv = q[:, :].rearrange("(n p) d -> p n d", p=P)    # [128, 32, 1024]
            ovw = out[:, :].rearrange("(n p) d -> p n d", p=P)
            for i in range(NT // TPB):
                qt = qin.tile([P, TPB, DM], F32)
                nc.sync.dma_start(out=qt[:, :, :],
                                  in_=qv[:, i * TPB:(i + 1) * TPB, :])
                ot = outp.tile([P, TPB, DM], F32)
                for j in range(TPB):
                    sub = qt[:, j, :]
                    nc.vector.tensor_add(sub, sub, tvb[:, :])
                    st = stats.tile([P, 2, 6], F32)
                    nc.vector.bn_stats(st[:, 0, :], qt[:, j, 0:512])
                    nc.vector.bn_stats(st[:, 1, :], qt[:, j, 512:1024])
                    mv = stats.tile([P, 2], F32)
                    nc.vector.bn_aggr(mv[:, :], st[:, :, :])
                    istd = stats.tile([P, 1], F32)
                    nc.scalar.activation(istd[:, :], mv[:, 1:2], AF.Sqrt,
                                         bias=eps_s[:, :], scale=1.0)
                    nc.vector.reciprocal(istd[:, :], istd[:, :])
                    nmu = stats.tile([P, 1], F32)
                    nc.vector.tensor_scalar(nmu[:, :], mv[:, 0:1], istd[:, :],
                                            -1.0, OP.mult, OP.mult)
                    nc.scalar.activation(ot[:, j, :], sub, AF.Identity,
                                         bias=nmu[:, :], scale=istd[:, :])
                    if apply_affine:
                        nc.vector.tensor_mul(ot[:, j, :], ot[:, j, :], gb[:, :])
                        nc.vector.tensor_add(ot[:, j, :], ot[:, j, :], bb[:, :])
                # out-DMA triggered from the scalar sequencer so the in/out
                # trigger streams don't serialize on one sequencer
                nc.scalar.dma_start(out=ovw[:, i * TPB:(i + 1) * TPB, :],
                                    in_=ot[:, :, :])

    nc.compile()
    return nc


_PROGRAM_CACHE = {}


def _get_program(apply_affine: bool):
    if apply_affine not in _PROGRAM_CACHE:
        _PROGRAM_CACHE[apply_affine] = _build_program(apply_affine)
    return _PROGRAM_CACHE[apply_affine]


def _pack_params(p):
    """[15, 64] per-node params -> [64, 240] kernel layout (levels packed,
    replicated across the 16 heads)."""
    parts = []
    for start, size in [(7, 8), (3, 4), (1, 2), (0, 1)]:
        parts.append(np.repeat(p[start:start + size].T, NH, axis=1))
    return np.ascontiguousarray(np.concatenate(parts, axis=1), dtype=np.float32)


def make_in_maps(query, key_value, W, bias, lif_th, lif_tau, lif_vreset,
                 ln_gamma, ln_beta, apply_affine):
    # fold the tree's pair-mean 0.5 into the inner-node (0..6) weights and
    # LIF thresholds/resets — exact power-of-two rescalings (see kernel)
    W2 = W.astype(np.float32).copy()
    W2[0:7] *= 0.5
    th2 = lif_th.astype(np.float32).copy()
    th2[0:7] *= 2.0
    vr2 = lif_vreset.astype(np.float32).copy()
    vr2[0:7] *= 2.0
    wT = np.ascontiguousarray(W2.transpose(2, 0, 1).reshape(HD, 15 * HD))
    thT = _pack_params(th2)
    tauT = _pack_params(lif_tau.astype(np.float32))
    vrT = _pack_params(vr2)
    bT = _pack_params(bias.astype(np.float32))
    id64 = np.eye(HD, dtype=np.float32)
    sel_np = np.zeros((NH, NH * P), dtype=np.float32)
    for h in range(NH):
        sel_np[h, h * P:(h + 1) * P] = 1.0
    in_maps = []
    for b in range(BATCH):
        xT = np.ascontiguousarray(
            key_value[b].astype(np.float32).reshape(16, NH, HD)[:8]
            .transpose(2, 0, 1).reshape(HD, 128))
        m = dict(q=np.ascontiguousarray(query[b], dtype=np.float32),
                 xT=xT, wT=wT, thT=thT, tauT=tauT, vrT=vrT, bT=bT, id64=id64,
                 sel=sel_np)
        if apply_affine:
            m["gam"] = np.ascontiguousarray(ln_gamma, dtype=np.float32)
            m["bet"] = np.ascontiguousarray(ln_beta, dtype=np.float32)
        in_maps.append(m)
    return in_maps


def kernel(query, key_value, W, bias, lif_th, lif_tau, lif_vreset,
           ln_gamma, ln_beta, **run_kwargs):
    query = np.asarray(query, dtype=np.float32)
    key_value = np.asarray(key_value, dtype=np.float32)
    W = np.asarray(W, dtype=np.float32)
    bias = np.asarray(bias, dtype=np.float32)
    lif_th = np.asarray(lif_th, dtype=np.float32)
    lif_tau = np.asarray(lif_tau, dtype=np.float32)
    lif_vreset = np.asarray(lif_vreset, dtype=np.float32)
    ln_gamma = np.asarray(ln_gamma, dtype=np.float32)
    ln_beta = np.asarray(ln_beta, dtype=np.float32)

    apply_affine = not (np.all(ln_gamma == 1.0) and np.all(ln_beta == 0.0))
    nc = _get_program(apply_affine)
    in_maps = make_in_maps(query, key_value, W, bias, lif_th, lif_tau,
                           lif_vreset, ln_gamma, ln_beta, apply_affine)
    res = run_bass_kernel_spmd(nc, in_maps, core_ids=list(range(N_CORES)),
                               **run_kwargs)
    global _LAST_RESULTS
    _LAST_RESULTS = res
    out = np.stack([res.results[b]["out"] for b in range(BATCH)], axis=0)
    return out


_LAST_RESULTS = None
